# revision 10
# baseline (speedup 1.0000x reference)
"""CapacitiveMHA Trainium2 kernel.

Sharding: 8 cores = 4 batch shards x 2 head-group (tensor-parallel) shards.
Each core handles (batch b, heads [8g, 8g+8)): router+topk+gather replicated
per batch shard; q/kv/out projections and SDPA tensor-parallel over heads;
each core scatters its partial out-projection over the full sequence and the
host sums the two head-group partials per batch.

Router scores run on the DVE (fused multiply-reduce over f32 q rows), so q
is streamed once in natural layout — no host-side transpose or bf16 split.
kv-projection runs on the PE concurrently with the router/top-k (emission
split), and all constants are packed into two blobs to minimize the number
of executable operands.
"""

import sys

sys.path.insert(0, "/opt/trn_rl_repo")

import numpy as np
import ml_dtypes

import concourse.bass as bass
import concourse.bacc as bacc
import concourse.mybir as mybir
import concourse.tile as tile
from concourse.bass_utils import run_bass_kernel_spmd

B, S, D = 4, 4096, 1024
H = 16          # total heads
HG = 2          # head groups (TP degree)
HC = H // HG    # heads per core = 8
dh = D // H     # 64
EH = D // HG    # e-range per core = 512
CAP = 512       # capacity
ROPE_BASE = 10000.0

dt = mybir.dt
F32, BF16, I32 = dt.float32, dt.bfloat16, dt.int32
F16 = dt.float16
AF = mybir.ActivationFunctionType
OP = mybir.AluOpType
X = mybir.AxisListType.X

# cblob (f32) column layout
C_IDENT = 0          # [128, 128] identity
C_U32 = 128          # [32, 32] upper-tri ones (inclusive)
C_L128 = 160         # [128, 128] strict upper-tri ones
C_ONES = 288         # [1, 128] ones row
C_REP64 = 416        # [64, 128] tiled eye(64) twice
C_RW = 544           # [128, 1024] router weights replicated across partitions
C_TOT = 1568

# cblob16 (f16) column layout
H_IOTAH = 0          # [128, 32] (128c+p)//64
H_IOTAL = 32         # [128, 32] (128c+p)%64
H_IOTA512 = 64       # [128, 512] iota row 0..511
H_FKREP = 576        # [128, 4096] k-side rope sin/cos (1/sqrt(d) folded in wk)
H_TOT = 4672


def _bf16(x):
    return np.asarray(x, dtype=ml_dtypes.bfloat16)


def _build_program():
    nc = bacc.Bacc()

    q_nat = nc.dram_tensor("q_nat", [S, D], F32, kind="ExternalInput")
    vT = nc.dram_tensor("vT", [D, S], BF16, kind="ExternalInput")
    wall = nc.dram_tensor("wall", [D, 2560], BF16, kind="ExternalInput")
    cblob = nc.dram_tensor("cblob", [128, C_TOT], F32, kind="ExternalInput")
    cblob16 = nc.dram_tensor("cblob16", [128, H_TOT], F16, kind="ExternalInput")
    fkT = nc.dram_tensor("fkT", [S, dh], F32, kind="ExternalInput")

    out_rows = nc.dram_tensor("out_rows", [CAP, D], F32, kind="ExternalOutput")
    out_idx = nc.dram_tensor("out_idx", [CAP, 1], F32, kind="ExternalOutput")

    with tile.TileContext(nc) as tc:
        _body(nc, tc, locals())
    nc.compile()
    return nc


def _body(nc, tc, t):
    q_nat, vT, wall, fkT = t["q_nat"], t["vT"], t["wall"], t["fkT"]
    cblob_d, cblob16_d = t["cblob"], t["cblob16"]
    out_rows, out_idx = t["out_rows"], t["out_idx"]

    with (
        tc.tile_pool(name="const", bufs=1) as cp,
        tc.tile_pool(name="persist", bufs=1) as pp,
        tc.tile_pool(name="scratch", bufs=2) as scr,
        tc.tile_pool(name="respool", bufs=1) as resp,
    ):
        # ---- constants (two packed blobs) ----
        cb = cp.tile([128, C_TOT], F32, name="cb", tag="cb")
        nc.gpsimd.dma_start(cb[:], cblob_d[:])
        cb16 = cp.tile([128, H_TOT], F16, name="cb16", tag="cb16")

        ident = cb[:, C_IDENT:C_IDENT + 128]
        u32c = cb[0:32, C_U32:C_U32 + 32]
        l128c = cb[:, C_L128:C_L128 + 128]
        ones1x128 = cb[0:1, C_ONES:C_ONES + 128]
        rep64c = cb[0:64, C_REP64:C_REP64 + 128]
        rw_rep = cb[:, C_RW:C_RW + 1024]
        iotah = cb16[:, H_IOTAH:H_IOTAH + 32]
        iotal = cb16[:, H_IOTAL:H_IOTAL + 32]
        iota512 = cb16[:, H_IOTA512:H_IOTA512 + 512]

        identb = cp.tile([128, 128], BF16, name="identb", tag="identb")
        nc.vector.tensor_copy(identb[:], ident)

        # ---- weights from the packed wall tensor ----
        wk_sb, wv_sb, wq_sb, ow_sb = [], [], [], []
        for d in range(8):
            rsl = slice(128 * d, 128 * (d + 1))
            tk = pp.tile([128, EH], BF16, name=f"wk{d}", tag=f"wk{d}")
            nc.gpsimd.dma_start(tk[:], wall[rsl, 0:512])
            wk_sb.append(tk)
            tv = pp.tile([128, EH], BF16, name=f"wv{d}", tag=f"wv{d}")
            nc.gpsimd.dma_start(tv[:], wall[rsl, 512:1024])
            wv_sb.append(tv)
            tq = pp.tile([128, EH], BF16, name=f"wq{d}", tag=f"wq{d}")
            nc.gpsimd.dma_start(tq[:], wall[rsl, 1024:1536])
            wq_sb.append(tq)
        for e in range(4):
            to = pp.tile([128, D], BF16, name=f"ow{e}", tag=f"ow{e}")
            nc.gpsimd.dma_start(to[:], wall[128 * e:128 * (e + 1), 1536:2560])
            ow_sb.append(to)
        nc.gpsimd.dma_start(cb16[:], cblob16_d[:])

        # persistent activations
        kT_sb = [pp.tile([128, S], BF16, name=f"kT{e}", tag=f"kT{e}") for e in range(4)]
        qT_sb = [pp.tile([128, 512], BF16, name=f"qTt{e}", tag=f"qTt{e}") for e in range(4)]
        att_sb = [pp.tile([128, 512], BF16, name=f"att{e}", tag=f"att{e}") for e in range(4)]
        v_sb = pp.tile([128, 520 * 32], BF16, name="v_sb", tag="v_sb")  # 32 chunks x (8h x 65)
        scores_cm = pp.tile([128, 32], F32, name="scores_cm", tag="scores_cm")

        # ===== Phase A: router scores on DVE (q streamed once, f32) =====
        ap_pools = tc.tile_pool(name="qstream", bufs=2)
        qsp = ap_pools.__enter__()
        prod_cm = tc.tile_pool(name="prodp", bufs=2)
        prodp = prod_cm.__enter__()
        for tq4 in range(32):
            qt = qsp.tile([128, D], F32, name="qt", tag="qt")
            nc.sync.dma_start(qt[:], q_nat[128 * tq4:128 * (tq4 + 1), :])
            prod = prodp.tile([128, D], F32, name="prod", tag="prod")
            nc.vector.affine_mul_reduce(
                out=prod[:], accum_out=scores_cm[:, tq4:tq4 + 1],
                in0=qt[:], in1=rw_rep, scale=1.0, bias=0.0,
            )

        # ===== Phase D (first half): kv-proj on PE, overlapped with A/B/C =====
        d_pools = tc.tile_pool(name="stream", bufs=12)
        streamp = d_pools.__enter__()
        pk_cm = tc.tile_pool(name="pk", bufs=2, space="PSUM")
        pkp = pk_cm.__enter__()
        pv_cm = tc.tile_pool(name="pv", bufs=2, space="PSUM")
        pvp = pv_cm.__enter__()

        def emit_kvproj(sc2_list):
            for sc2 in sc2_list:
                vblk = []
                for d in range(8):
                    blk = streamp.tile([128, 1024], BF16, name="stream",
                                       tag="stream")
                    nc.scalar.dma_start(
                        blk[:],
                        vT[128 * d:128 * (d + 1), 1024 * sc2:1024 * (sc2 + 1)],
                    )
                    vblk.append(blk)
                for sch in range(2):
                    sc = 2 * sc2 + sch
                    for e in range(4):
                        pk = pkp.tile([128, 512], F32)
                        for d in range(8):
                            nc.tensor.matmul(
                                pk[:], lhsT=wk_sb[d][:, 128 * e:128 * (e + 1)],
                                rhs=vblk[d][:, 512 * sch:512 * (sch + 1)],
                                start=(d == 0), stop=(d == 7),
                            )
                        nc.vector.tensor_tensor(
                            kT_sb[e][:, 512 * sc:512 * (sc + 1)], pk[:],
                            cb16[:, H_FKREP + 512 * sc:H_FKREP + 512 * (sc + 1)],
                            op=OP.mult,
                        )
                    for q4 in range(4):
                        pv = pvp.tile([128, 512], F32)
                        for d in range(8):
                            nc.tensor.matmul(
                                pv[:],
                                lhsT=vblk[d][:, 512 * sch + 128 * q4:
                                             512 * sch + 128 * (q4 + 1)],
                                rhs=wv_sb[d][:], start=(d == 0), stop=(d == 7),
                            )
                        base = 520 * (4 * sc + q4)
                        nc.vector.tensor_copy(
                            v_sb[:, base:base + 520]
                            .rearrange("p (h c) -> p h c", h=8)[:, :, 0:64],
                            pv[:].rearrange("p (h c) -> p h c", h=8),
                        )
                        nc.vector.memset(
                            v_sb[:, base:base + 520]
                            .rearrange("p (h c) -> p h c", h=8)[:, :, 64:65],
                            1.0,
                        )

        emit_kvproj([0, 1])

        # ===== Phase B: top-512 threshold + compaction =====
        with tc.tile_pool(name="pb", bufs=1, space="PSUM") as pbp:
            kth = scr.tile([1, 2], F32, name="kth", tag="kth")
            # quantile s.t. k_adj = floor((1-q)*4095) = 510 -> out[0,1] =
            # desc[511] = the 512th-largest score = selection threshold
            nc.gpsimd.kth_largest(
                kth[:], scores_cm[:], 32, 510, quantile=1.0 - 510.5 / 4095.0
            )
            lo = kth[:, 1:2]

            # threshold column (128,1) via PE outer
            pthr = pbp.tile([128, 1], F32, name="pthr", tag="pb")
            nc.tensor.matmul(pthr[:], lhsT=ones1x128, rhs=lo[:],
                             start=True, stop=True)
            thr_col = scr.tile([128, 1], F32, name="thr_col", tag="thr_col")
            nc.vector.tensor_copy(thr_col[:], pthr[:])

            # masks (mask in (128,32) and transposed (32,128))
            mask = pp.tile([128, 32], F32, name="mask", tag="mask")
            nc.vector.tensor_scalar(mask[:], scores_cm[:], thr_col[:], None,
                                    op0=OP.is_ge)
            pst = pbp.tile([32, 128], F32, name="pst", tag="pb")
            nc.tensor.transpose(pst[:], scores_cm[:], ident)
            s_T = scr.tile([32, 128], F32, name="s_T", tag="s_T")
            nc.vector.tensor_copy(s_T[:], pst[:])
            mask_T = scr.tile([32, 128], F32, name="mask_T", tag="mask_T")
            nc.vector.tensor_scalar(mask_T[:], s_T[:], thr_col[:32, :], None,
                                    op0=OP.is_ge)

            # in-row inclusive prefix: pfx = mask_T.T @ U32  -> (128, 32)
            ppfx = pbp.tile([128, 32], F32, name="ppfx", tag="pb")
            nc.tensor.matmul(ppfx[:], lhsT=mask_T[:], rhs=u32c,
                             start=True, stop=True)
            pfx = scr.tile([128, 32], F32, name="pfx", tag="pfx")
            nc.vector.tensor_copy(pfx[:], ppfx[:])
            # cross-row exclusive prefix of row sums: S = L128.T @ rowsum
            pS = pbp.tile([128, 1], F32, name="pS", tag="pb")
            nc.tensor.matmul(pS[:], lhsT=l128c, rhs=pfx[:, 31:32],
                             start=True, stop=True)
            Scol = scr.tile([128, 1], F32, name="Scol", tag="Scol")
            nc.vector.tensor_copy(Scol[:], pS[:])

            rank = scr.tile([128, 32], F32, name="rank", tag="rank")
            nc.vector.tensor_tensor(rank[:], pfx[:], Scol[:].to_broadcast([128, 32]),
                                    op=OP.add)
            nc.vector.tensor_tensor(rank[:], rank[:], mask[:], op=OP.subtract)
            # rank_eff = mask ? rank : 512  (= (rank-512)*mask + 512)
            nc.vector.tensor_scalar(rank[:], rank[:], float(CAP), None,
                                    op0=OP.subtract)
            nc.vector.tensor_tensor(rank[:], rank[:], mask[:], op=OP.mult)
            nc.vector.tensor_scalar(rank[:], rank[:], float(CAP), None, op0=OP.add)

            # fp16 split of scores: s = shi + slo (each fp16-exact)
            shi = scr.tile([128, 32], F16, name="shi", tag="shi")
            nc.vector.tensor_copy(shi[:], scores_cm[:])
            slo = scr.tile([128, 32], F16, name="slo", tag="slo")
            nc.vector.tensor_tensor(slo[:], scores_cm[:], shi[:], op=OP.subtract)
            # fp16 rank copy for 2x-mode one-hot compares
            rank16 = scr.tile([128, 32], F16, name="rank16", tag="rank16")
            nc.vector.tensor_copy(rank16[:], rank[:])

            # combo tile: interleave [ihi | ilo | shi | slo] quads -> (128, 128)
            combo = scr.tile([128, 128], F16, name="combo", tag="combo")
            for ci, srct in enumerate((iotah, iotal, shi[:], slo[:])):
                nc.vector.tensor_copy(
                    combo[:].rearrange("p (c four) -> p c four", four=4)
                    [:, :, ci:ci + 1],
                    srct.rearrange("p (c one) -> p c one", one=1),
                )

            # one-hot P tiles + [idx_hi; idx_lo; w_hi; w_lo] extraction
            piw = pbp.tile([4, 512], F32, name="piw", tag="pb")
            for c in range(32):
                Pc = scr.tile([128, 512], F16, name="Pc", tag="Pc")
                nc.vector.tensor_tensor(
                    Pc[:], rank16[:, c:c + 1].to_broadcast([128, 512]), iota512,
                    op=OP.is_equal,
                )
                nc.tensor.matmul(piw[:], lhsT=combo[:, 4 * c:4 * c + 4], rhs=Pc[:],
                                 start=(c == 0), stop=(c == 31))
            iw_sb = scr.tile([4, 512], F32, name="iw_sb", tag="iw_sb")
            nc.vector.tensor_copy(iw_sb[:], piw[:])

            # transpose to column layout (4 chunks of 128)
            idx_col = []
            topw_col = []
            for j in range(4):
                pt = pbp.tile([128, 4], F32, name="pt", tag="pb")
                nc.tensor.transpose(pt[:], iw_sb[:, 128 * j:128 * (j + 1)],
                                    ident[:4, :4])
                iwT = pp.tile([128, 4], F32, name=f"iwT{j}", tag=f"iwT{j}")
                nc.vector.tensor_copy(iwT[:], pt[:])
                # idx = 64*hi + lo ; w = whi + wlo
                nc.vector.tensor_scalar(iwT[:, 0:1], iwT[:, 0:1], 64.0, None,
                                        op0=OP.mult)
                nc.vector.tensor_tensor(iwT[:, 0:1], iwT[:, 0:1], iwT[:, 1:2],
                                        op=OP.add)
                nc.vector.tensor_tensor(iwT[:, 2:3], iwT[:, 2:3], iwT[:, 3:4],
                                        op=OP.add)
                ic = pp.tile([128, 1], I32, name=f"idxc{j}", tag=f"idxc{j}")
                nc.vector.tensor_copy(ic[:], iwT[:, 0:1])
                nc.gpsimd.dma_start(out_idx[128 * j:128 * (j + 1), :],
                                    iwT[:, 0:1])
                idx_col.append(ic)
                topw_col.append(iwT)

        # ===== Phase C: gather + q-proj + rope-q =====
        with tc.tile_pool(name="pc", bufs=2, space="PSUM") as pcp:
            # critical path first: gather query rows (cast to bf16 in SWDGE)
            res = [resp.tile([128, D], BF16, name=f"res{j}", tag=f"res{j}") for j in range(4)]
            for j in range(4):
                nc.gpsimd.indirect_dma_start(
                    out=res[j][:], out_offset=None, in_=q_nat[:],
                    in_offset=bass.IndirectOffsetOnAxis(ap=idx_col[j][:, 0:1], axis=0),
                )
            # transpose resampled -> rT (d-part, c-free), bf16
            rT_sb = []
            for d in range(8):
                prt = pcp.tile([128, 512], BF16, name="prt", tag="pc")
                for j in range(4):
                    nc.tensor.transpose(
                        prt[:, 128 * j:128 * (j + 1)],
                        res[j][:, 128 * d:128 * (d + 1)], identb[:],
                    )
                rt = pp.tile([128, 512], BF16, name=f"rT{d}", tag=f"rT{d}")
                nc.vector.tensor_copy(rt[:], prt[:])
                rT_sb.append(rt)

            # rope-q factor: gather fkT rows then transpose into (128, 512)
            pfq = pcp.tile([64, 512], F32, name="pfq", tag="pc")
            for j in range(4):
                fqg = scr.tile([128, 64], F32, name="fqg", tag="fqg")
                nc.gpsimd.indirect_dma_start(
                    out=fqg[:], out_offset=None, in_=fkT[:],
                    in_offset=bass.IndirectOffsetOnAxis(ap=idx_col[j][:, 0:1], axis=0),
                )
                nc.tensor.transpose(pfq[:, 128 * j:128 * (j + 1)], fqg[:],
                                    ident)
            fq_half = scr.tile([64, 512], F32, name="fq_half", tag="fq_half")
            nc.vector.tensor_copy(fq_half[:], pfq[:])
            pfq2 = pcp.tile([128, 512], F32, name="pfq2", tag="pc")
            nc.tensor.matmul(pfq2[:], lhsT=rep64c, rhs=fq_half[:],
                             start=True, stop=True)
            fq_rep = pp.tile([128, 512], F32, name="fq_rep", tag="fq_rep")
            nc.vector.tensor_copy(fq_rep[:], pfq2[:])

            # q-proj (+rope) -> qT_sb
            for e in range(4):
                pq = pcp.tile([128, 512], F32, name="pq", tag="pc")
                for d in range(8):
                    nc.tensor.matmul(
                        pq[:], lhsT=wq_sb[d][:, 128 * e:128 * (e + 1)],
                        rhs=rT_sb[d][:], start=(d == 0), stop=(d == 7),
                    )
                nc.vector.tensor_tensor(qT_sb[e][:], pq[:], fq_rep[:], op=OP.mult)

        # ===== Phase D (second half) =====
        emit_kvproj([2, 3])

        pv_cm.__exit__(None, None, None)
        pk_cm.__exit__(None, None, None)
        d_pools.__exit__(None, None, None)
        prod_cm.__exit__(None, None, None)
        ap_pools.__exit__(None, None, None)

        # ===== Phase E: SDPA (4 waves of 2 heads) =====
        with (
            tc.tile_pool(name="psc", bufs=2, space="PSUM") as pscp,
            tc.tile_pool(name="patt", bufs=2, space="PSUM") as pattp,
            tc.tile_pool(name="epool", bufs=4) as ep,
        ):
            for e in range(4):
                patt = [pattp.tile([65, 512], F32, name=f"patt{hh}", tag=f"patt{hh}") for hh in range(2)]
                for tch in range(32):
                    psc = pscp.tile([128, 1024], F32)
                    for hh in range(2):
                        nc.tensor.matmul(
                            psc[:, 512 * hh:512 * (hh + 1)],
                            lhsT=kT_sb[e][64 * hh:64 * (hh + 1),
                                          128 * tch:128 * (tch + 1)],
                            rhs=qT_sb[e][64 * hh:64 * (hh + 1), :],
                            start=True, stop=True,
                        )
                    et = ep.tile([128, 1024], BF16, name="et", tag="et")
                    nc.scalar.activation(et[:], psc[:], AF.Exp)
                    for hh in range(2):
                        vb = 520 * tch + 65 * (2 * e + hh)
                        nc.tensor.matmul(
                            patt[hh][:],
                            lhsT=v_sb[:, vb:vb + 65],
                            rhs=et[:, 512 * hh:512 * (hh + 1)],
                            start=(tch == 0), stop=(tch == 31),
                        )
                for hh in range(2):
                    recip = scr.tile([1, 512], F32, name="recip", tag="recip", bufs=1)
                    nc.vector.reciprocal(recip[:], patt[hh][64:65, :])
                    rrep = scr.tile([64, 512], F32, name="rrep", tag="rrep", bufs=1)
                    nc.gpsimd.partition_broadcast(rrep[:], recip[:], channels=64)
                    nc.vector.tensor_tensor(
                        att_sb[e][64 * hh:64 * (hh + 1), :],
                        patt[hh][0:64, :], rrep[:], op=OP.mult,
                    )

        # ===== Phase F: out-proj + scale + scatter =====
        with (
            tc.tile_pool(name="po", bufs=2, space="PSUM") as pop,
            tc.tile_pool(name="opool", bufs=2) as op_,
        ):
            for j in range(4):
                po = pop.tile([128, 1024], F32)
                for e in range(4):
                    for k in range(2):
                        nc.tensor.matmul(
                            po[:, 512 * k:512 * (k + 1)],
                            lhsT=att_sb[e][:, 128 * j:128 * (j + 1)],
                            rhs=ow_sb[e][:, 512 * k:512 * (k + 1)],
                            start=(e == 0), stop=(e == 3),
                        )
                osb = op_.tile([128, 1024], F32, name="osb", tag="osb")
                nc.scalar.mul(osb[:], po[:], topw_col[j][:, 2:3])
                nc.sync.dma_start(out_rows[128 * j:128 * (j + 1), :], osb[:])


_NC_CACHE = None


def _get_nc():
    global _NC_CACHE
    if _NC_CACHE is None:
        _NC_CACHE = _build_program()
    return _NC_CACHE


def _host_constants():
    pos = np.arange(S, dtype=np.float32)
    freqs = np.exp(
        np.linspace(0.0, -1.0, dh // 2, dtype=np.float32)
        * np.log(np.float32(ROPE_BASE))
    ).astype(np.float32)
    angles = pos[:, None] * freqs[None, :]          # (S, 32) f32
    fkT = np.concatenate([np.sin(angles), np.cos(angles)], axis=1).astype(
        np.float32
    )                                               # (S, 64)
    fkrep = np.concatenate([fkT.T, fkT.T], axis=0)  # (128, S), pure sin/cos

    p = np.arange(128)[:, None]
    c = np.arange(32)[None, :]
    iota_cm = (128 * c + p).astype(np.float32)

    cblob = np.zeros((128, C_TOT), np.float32)
    cblob[:, C_IDENT:C_IDENT + 128] = np.eye(128, dtype=np.float32)
    cblob[0:32, C_U32:C_U32 + 32] = np.triu(np.ones((32, 32), np.float32))
    cblob[:, C_L128:C_L128 + 128] = np.triu(np.ones((128, 128), np.float32), k=1)
    cblob[0:1, C_ONES:C_ONES + 128] = 1.0
    cblob[0:64, C_REP64:C_REP64 + 128] = np.tile(np.eye(64, dtype=np.float32), (1, 2))

    cblob16 = np.zeros((128, H_TOT), np.float16)
    cblob16[:, H_IOTAH:H_IOTAH + 32] = (iota_cm // 64).astype(np.float16)
    cblob16[:, H_IOTAL:H_IOTAL + 32] = (iota_cm % 64).astype(np.float16)
    cblob16[:, H_IOTA512:H_IOTA512 + 512] = np.tile(
        np.arange(512, dtype=np.float16)[None, :], (128, 1)
    )
    cblob16[:, H_FKREP:H_FKREP + S] = fkrep.astype(np.float16)

    return fkT, cblob, cblob16


def make_in_maps(query_seq, value_seq, router_w, q_w, kv_w, out_w):
    query_seq = np.asarray(query_seq, np.float32)
    value_seq = np.asarray(value_seq, np.float32)
    router_w = np.asarray(router_w, np.float32)
    q_w = np.asarray(q_w, np.float32)
    kv_w = np.asarray(kv_w, np.float32)
    out_w = np.asarray(out_w, np.float32)

    fkT, cblob_base, cblob16 = _host_constants()

    vTs = [_bf16(np.ascontiguousarray(value_seq[b].T)) for b in range(B)]

    walls = []
    for g in range(HG):
        es = slice(EH * g, EH * (g + 1))
        wall = np.zeros((D, 2560), np.float32)
        wall[:, 0:512] = kv_w[es, :].T / 8.0          # 1/sqrt(dh) folded in wk
        wall[:, 512:1024] = kv_w[D + EH * g:D + EH * (g + 1), :].T
        wall[:, 1024:1536] = q_w[es, :].T
        wall[0:512, 1536:2560] = out_w[:, es].T
        walls.append(_bf16(wall))

    cblob = cblob_base.copy()
    cblob[:, C_RW:C_RW + 1024] = router_w.reshape(1, D)

    in_maps = []
    for core in range(8):
        b, g = core // 2, core % 2
        m = dict(
            q_nat=np.ascontiguousarray(query_seq[b]),
            vT=vTs[b],
            wall=walls[g],
            cblob=cblob,
            cblob16=cblob16,
            fkT=fkT,
        )
        in_maps.append(m)
    return in_maps


def assemble_output(outs_by_name):
    """Full (B,S,D) output from per-core results concatenated on axis 0.

    Cores 2b, 2b+1 hold batch b's two head-group partials of the 512
    selected rows (identical selection), scattered into a zero background
    on the host."""
    rows = np.asarray(outs_by_name["out_rows"], dtype=np.float32)
    idxf = np.asarray(outs_by_name["out_idx"], dtype=np.float32)
    out = np.zeros((B, S, D), np.float32)
    for b in range(B):
        r0 = rows[(2 * b) * CAP:(2 * b + 1) * CAP]
        r1 = rows[(2 * b + 1) * CAP:(2 * b + 2) * CAP]
        idx = idxf[(2 * b) * CAP:(2 * b + 1) * CAP, 0].astype(np.int64)
        out[b, idx] = r0 + r1
    return out


def kernel(query_seq, value_seq, router_w, q_w, kv_w, out_w):
    nc = _get_nc()
    in_maps = make_in_maps(query_seq, value_seq, router_w, q_w, kv_w, out_w)
    try:
        res = run_bass_kernel_spmd(nc, in_maps, list(range(8))).results
    except Exception:
        # transient NRT_EXEC_UNIT_UNRECOVERABLE from a prior wedged session
        # clears on the next dispatch; retry once
        res = run_bass_kernel_spmd(nc, in_maps, list(range(8))).results
    out = np.zeros((B, S, D), np.float32)
    for b in range(B):
        idx = np.asarray(res[2 * b]["out_idx"], np.float32)[:, 0].astype(np.int64)
        out[b, idx] = (
            np.asarray(res[2 * b]["out_rows"], np.float32)
            + np.asarray(res[2 * b + 1]["out_rows"], np.float32)
        )
    return out


# revision 11
# speedup vs baseline: 1.4749x; 1.4749x over previous
"""CapacitiveMHA Trainium2 kernel.

Sharding: 8 cores = 4 batch shards x 2 head-group (tensor-parallel) shards.
Each core handles (batch b, heads [8g, 8g+8)): router+topk+gather replicated
per batch shard; q/kv/out projections and SDPA tensor-parallel over heads;
each core scatters its partial out-projection over the full sequence and the
host sums the two head-group partials per batch.

Router scores run on the DVE (fused multiply-reduce over f32 q rows), so q
is streamed once in natural layout — no host-side transpose or bf16 split.
kv-projection runs on the PE concurrently with the router/top-k (emission
split), and all constants are packed into two blobs to minimize the number
of executable operands.
"""

import sys

sys.path.insert(0, "/opt/trn_rl_repo")

import numpy as np
import ml_dtypes

import concourse.bass as bass
import concourse.bacc as bacc
import concourse.mybir as mybir
import concourse.tile as tile
from concourse.bass_utils import run_bass_kernel_spmd

B, S, D = 4, 4096, 1024
H = 16          # total heads
HG = 2          # head groups (TP degree)
HC = H // HG    # heads per core = 8
dh = D // H     # 64
EH = D // HG    # e-range per core = 512
CAP = 512       # capacity
ROPE_BASE = 10000.0

dt = mybir.dt
F32, BF16, I32 = dt.float32, dt.bfloat16, dt.int32
F16 = dt.float16
AF = mybir.ActivationFunctionType
OP = mybir.AluOpType
X = mybir.AxisListType.X

# cblob (f32) column layout
C_IDENT = 0          # [128, 128] identity
C_U32 = 128          # [32, 32] upper-tri ones (inclusive)
C_L128 = 160         # [128, 128] strict upper-tri ones
C_ONES = 288         # [1, 128] ones row
C_REP64 = 416        # [64, 128] tiled eye(64) twice
C_RW = 544           # [128, 1024] router weights replicated across partitions
C_TOT = 1568

# cblob16 (f16) column layout
H_IOTAH = 0          # [128, 32] (128c+p)//64
H_IOTAL = 32         # [128, 32] (128c+p)%64
H_IOTA512 = 64       # [128, 512] iota row 0..511
H_FKREP = 576        # [128, 4096] k-side rope sin/cos (1/sqrt(d) folded in wk)
H_TOT = 4672


def _bf16(x):
    return np.asarray(x, dtype=ml_dtypes.bfloat16)


def _build_program():
    nc = bacc.Bacc()

    q_nat = nc.dram_tensor("q_nat", [S, D], F32, kind="ExternalInput")
    vT = nc.dram_tensor("vT", [D, S], BF16, kind="ExternalInput")
    wall = nc.dram_tensor("wall", [D, 2560], BF16, kind="ExternalInput")
    cblob = nc.dram_tensor("cblob", [128, C_TOT], F32, kind="ExternalInput")
    cblob16 = nc.dram_tensor("cblob16", [128, H_TOT], F16, kind="ExternalInput")
    fkT = nc.dram_tensor("fkT", [S, dh], F32, kind="ExternalInput")

    out_rows = nc.dram_tensor("out_rows", [CAP, D + 8], F32, kind="ExternalOutput")

    with tile.TileContext(nc) as tc:
        _body(nc, tc, locals())
    nc.compile()
    return nc


def _body(nc, tc, t):
    q_nat, vT, wall, fkT = t["q_nat"], t["vT"], t["wall"], t["fkT"]
    cblob_d, cblob16_d = t["cblob"], t["cblob16"]
    out_rows = t["out_rows"]

    with (
        tc.tile_pool(name="const", bufs=1) as cp,
        tc.tile_pool(name="persist", bufs=1) as pp,
        tc.tile_pool(name="scratch", bufs=2) as scr,
        tc.tile_pool(name="respool", bufs=1) as resp,
    ):
        # ---- constants (two packed blobs) ----
        cb = cp.tile([128, C_TOT], F32, name="cb", tag="cb")
        nc.gpsimd.dma_start(cb[:], cblob_d[:])
        cb16 = cp.tile([128, H_TOT], F16, name="cb16", tag="cb16")

        ident = cb[:, C_IDENT:C_IDENT + 128]
        u32c = cb[0:32, C_U32:C_U32 + 32]
        l128c = cb[:, C_L128:C_L128 + 128]
        ones1x128 = cb[0:1, C_ONES:C_ONES + 128]
        rep64c = cb[0:64, C_REP64:C_REP64 + 128]
        rw_rep = cb[:, C_RW:C_RW + 1024]
        iotah = cb16[:, H_IOTAH:H_IOTAH + 32]
        iotal = cb16[:, H_IOTAL:H_IOTAL + 32]
        iota512 = cb16[:, H_IOTA512:H_IOTA512 + 512]

        identb = cp.tile([128, 128], BF16, name="identb", tag="identb")
        nc.vector.tensor_copy(identb[:], ident)

        # ---- weights from the packed wall tensor ----
        wk_sb, wv_sb, wq_sb, ow_sb = [], [], [], []
        for d in range(8):
            rsl = slice(128 * d, 128 * (d + 1))
            tk = pp.tile([128, EH], BF16, name=f"wk{d}", tag=f"wk{d}")
            nc.gpsimd.dma_start(tk[:], wall[rsl, 0:512])
            wk_sb.append(tk)
            tv = pp.tile([128, EH], BF16, name=f"wv{d}", tag=f"wv{d}")
            nc.gpsimd.dma_start(tv[:], wall[rsl, 512:1024])
            wv_sb.append(tv)
            tq = pp.tile([128, EH], BF16, name=f"wq{d}", tag=f"wq{d}")
            nc.gpsimd.dma_start(tq[:], wall[rsl, 1024:1536])
            wq_sb.append(tq)
        for e in range(4):
            to = pp.tile([128, D], BF16, name=f"ow{e}", tag=f"ow{e}")
            nc.gpsimd.dma_start(to[:], wall[128 * e:128 * (e + 1), 1536:2560])
            ow_sb.append(to)
        nc.gpsimd.dma_start(cb16[:], cblob16_d[:])

        # persistent activations
        kT_sb = [pp.tile([128, S], BF16, name=f"kT{e}", tag=f"kT{e}") for e in range(4)]
        qT_sb = [pp.tile([128, 512], BF16, name=f"qTt{e}", tag=f"qTt{e}") for e in range(4)]
        att_sb = [pp.tile([128, 512], BF16, name=f"att{e}", tag=f"att{e}") for e in range(4)]
        v_sb = pp.tile([128, 520 * 32], BF16, name="v_sb", tag="v_sb")  # 32 chunks x (8h x 65)
        scores_cm = pp.tile([128, 32], F32, name="scores_cm", tag="scores_cm")

        # ===== Phase A: router scores on DVE (q streamed once, f32) =====
        ap_pools = tc.tile_pool(name="qstream", bufs=2)
        qsp = ap_pools.__enter__()
        prod_cm = tc.tile_pool(name="prodp", bufs=2)
        prodp = prod_cm.__enter__()
        for tq4 in range(32):
            qt = qsp.tile([128, D], F32, name="qt", tag="qt")
            nc.sync.dma_start(qt[:], q_nat[128 * tq4:128 * (tq4 + 1), :])
            prod = prodp.tile([128, D], F32, name="prod", tag="prod")
            nc.vector.affine_mul_reduce(
                out=prod[:], accum_out=scores_cm[:, tq4:tq4 + 1],
                in0=qt[:], in1=rw_rep, scale=1.0, bias=0.0,
            )

        # ===== Phase D (first half): kv-proj on PE, overlapped with A/B/C =====
        d_pools = tc.tile_pool(name="stream", bufs=12)
        streamp = d_pools.__enter__()
        pk_cm = tc.tile_pool(name="pk", bufs=2, space="PSUM")
        pkp = pk_cm.__enter__()
        pv_cm = tc.tile_pool(name="pv", bufs=2, space="PSUM")
        pvp = pv_cm.__enter__()

        def emit_kvproj(sc2_list):
            for sc2 in sc2_list:
                vblk = []
                for d in range(8):
                    blk = streamp.tile([128, 1024], BF16, name="stream",
                                       tag="stream")
                    nc.scalar.dma_start(
                        blk[:],
                        vT[128 * d:128 * (d + 1), 1024 * sc2:1024 * (sc2 + 1)],
                    )
                    vblk.append(blk)
                for sch in range(2):
                    sc = 2 * sc2 + sch
                    for e in range(4):
                        pk = pkp.tile([128, 512], F32)
                        for d in range(8):
                            nc.tensor.matmul(
                                pk[:], lhsT=wk_sb[d][:, 128 * e:128 * (e + 1)],
                                rhs=vblk[d][:, 512 * sch:512 * (sch + 1)],
                                start=(d == 0), stop=(d == 7),
                            )
                        nc.vector.tensor_tensor(
                            kT_sb[e][:, 512 * sc:512 * (sc + 1)], pk[:],
                            cb16[:, H_FKREP + 512 * sc:H_FKREP + 512 * (sc + 1)],
                            op=OP.mult,
                        )
                    for q4 in range(4):
                        pv = pvp.tile([128, 512], F32)
                        for d in range(8):
                            nc.tensor.matmul(
                                pv[:],
                                lhsT=vblk[d][:, 512 * sch + 128 * q4:
                                             512 * sch + 128 * (q4 + 1)],
                                rhs=wv_sb[d][:], start=(d == 0), stop=(d == 7),
                            )
                        base = 520 * (4 * sc + q4)
                        nc.vector.tensor_copy(
                            v_sb[:, base:base + 520]
                            .rearrange("p (h c) -> p h c", h=8)[:, :, 0:64],
                            pv[:].rearrange("p (h c) -> p h c", h=8),
                        )
                        nc.vector.memset(
                            v_sb[:, base:base + 520]
                            .rearrange("p (h c) -> p h c", h=8)[:, :, 64:65],
                            1.0,
                        )

        emit_kvproj([0, 1])

        # ===== Phase B: top-512 threshold + compaction =====
        with tc.tile_pool(name="pb", bufs=1, space="PSUM") as pbp:
            kth = scr.tile([1, 2], F32, name="kth", tag="kth")
            # quantile s.t. k_adj = floor((1-q)*4095) = 510 -> out[0,1] =
            # desc[511] = the 512th-largest score = selection threshold
            nc.gpsimd.kth_largest(
                kth[:], scores_cm[:], 32, 510, quantile=1.0 - 510.5 / 4095.0
            )
            lo = kth[:, 1:2]

            # threshold column (128,1) via PE outer
            pthr = pbp.tile([128, 1], F32, name="pthr", tag="pb")
            nc.tensor.matmul(pthr[:], lhsT=ones1x128, rhs=lo[:],
                             start=True, stop=True)
            thr_col = scr.tile([128, 1], F32, name="thr_col", tag="thr_col")
            nc.vector.tensor_copy(thr_col[:], pthr[:])

            # masks (mask in (128,32) and transposed (32,128))
            mask = pp.tile([128, 32], F32, name="mask", tag="mask")
            nc.vector.tensor_scalar(mask[:], scores_cm[:], thr_col[:], None,
                                    op0=OP.is_ge)
            pst = pbp.tile([32, 128], F32, name="pst", tag="pb")
            nc.tensor.transpose(pst[:], scores_cm[:], ident)
            s_T = scr.tile([32, 128], F32, name="s_T", tag="s_T")
            nc.vector.tensor_copy(s_T[:], pst[:])
            mask_T = scr.tile([32, 128], F32, name="mask_T", tag="mask_T")
            nc.vector.tensor_scalar(mask_T[:], s_T[:], thr_col[:32, :], None,
                                    op0=OP.is_ge)

            # in-row inclusive prefix: pfx = mask_T.T @ U32  -> (128, 32)
            ppfx = pbp.tile([128, 32], F32, name="ppfx", tag="pb")
            nc.tensor.matmul(ppfx[:], lhsT=mask_T[:], rhs=u32c,
                             start=True, stop=True)
            pfx = scr.tile([128, 32], F32, name="pfx", tag="pfx")
            nc.vector.tensor_copy(pfx[:], ppfx[:])
            # cross-row exclusive prefix of row sums: S = L128.T @ rowsum
            pS = pbp.tile([128, 1], F32, name="pS", tag="pb")
            nc.tensor.matmul(pS[:], lhsT=l128c, rhs=pfx[:, 31:32],
                             start=True, stop=True)
            Scol = scr.tile([128, 1], F32, name="Scol", tag="Scol")
            nc.vector.tensor_copy(Scol[:], pS[:])

            rank = scr.tile([128, 32], F32, name="rank", tag="rank")
            nc.vector.tensor_tensor(rank[:], pfx[:], Scol[:].to_broadcast([128, 32]),
                                    op=OP.add)
            nc.vector.tensor_tensor(rank[:], rank[:], mask[:], op=OP.subtract)
            # rank_eff = mask ? rank : 512  (= (rank-512)*mask + 512)
            nc.vector.tensor_scalar(rank[:], rank[:], float(CAP), None,
                                    op0=OP.subtract)
            nc.vector.tensor_tensor(rank[:], rank[:], mask[:], op=OP.mult)
            nc.vector.tensor_scalar(rank[:], rank[:], float(CAP), None, op0=OP.add)

            # fp16 split of scores: s = shi + slo (each fp16-exact)
            shi = scr.tile([128, 32], F16, name="shi", tag="shi")
            nc.vector.tensor_copy(shi[:], scores_cm[:])
            slo = scr.tile([128, 32], F16, name="slo", tag="slo")
            nc.vector.tensor_tensor(slo[:], scores_cm[:], shi[:], op=OP.subtract)
            # fp16 rank copy for 2x-mode one-hot compares
            rank16 = scr.tile([128, 32], F16, name="rank16", tag="rank16")
            nc.vector.tensor_copy(rank16[:], rank[:])

            # combo tile: interleave [ihi | ilo | shi | slo] quads -> (128, 128)
            combo = scr.tile([128, 128], F16, name="combo", tag="combo")
            for ci, srct in enumerate((iotah, iotal, shi[:], slo[:])):
                nc.vector.tensor_copy(
                    combo[:].rearrange("p (c four) -> p c four", four=4)
                    [:, :, ci:ci + 1],
                    srct.rearrange("p (c one) -> p c one", one=1),
                )

            # one-hot P tiles + [idx_hi; idx_lo; w_hi; w_lo] extraction
            piw = pbp.tile([4, 512], F32, name="piw", tag="pb")
            for c in range(32):
                Pc = scr.tile([128, 512], F16, name="Pc", tag="Pc")
                nc.vector.tensor_tensor(
                    Pc[:], rank16[:, c:c + 1].to_broadcast([128, 512]), iota512,
                    op=OP.is_equal,
                )
                nc.tensor.matmul(piw[:], lhsT=combo[:, 4 * c:4 * c + 4], rhs=Pc[:],
                                 start=(c == 0), stop=(c == 31))
            iw_sb = scr.tile([4, 512], F32, name="iw_sb", tag="iw_sb")
            nc.vector.tensor_copy(iw_sb[:], piw[:])

            # transpose to column layout (4 chunks of 128)
            idx_col = []
            topw_col = []
            for j in range(4):
                pt = pbp.tile([128, 4], F32, name="pt", tag="pb")
                nc.tensor.transpose(pt[:], iw_sb[:, 128 * j:128 * (j + 1)],
                                    ident[:4, :4])
                iwT = pp.tile([128, 4], F32, name=f"iwT{j}", tag=f"iwT{j}")
                nc.vector.tensor_copy(iwT[:], pt[:])
                # idx = 64*hi + lo ; w = whi + wlo
                nc.vector.tensor_scalar(iwT[:, 0:1], iwT[:, 0:1], 64.0, None,
                                        op0=OP.mult)
                nc.vector.tensor_tensor(iwT[:, 0:1], iwT[:, 0:1], iwT[:, 1:2],
                                        op=OP.add)
                nc.vector.tensor_tensor(iwT[:, 2:3], iwT[:, 2:3], iwT[:, 3:4],
                                        op=OP.add)
                ic = pp.tile([128, 1], I32, name=f"idxc{j}", tag=f"idxc{j}")
                nc.vector.tensor_copy(ic[:], iwT[:, 0:1])
                nc.gpsimd.dma_start(
                    out_rows[128 * j:128 * (j + 1), D:D + 1], iwT[:, 0:1])
                idx_col.append(ic)
                topw_col.append(iwT)

        # ===== Phase C: gather + q-proj + rope-q =====
        with tc.tile_pool(name="pc", bufs=2, space="PSUM") as pcp:
            # critical path first: gather query rows (cast to bf16 in SWDGE)
            res = [resp.tile([128, D], BF16, name=f"res{j}", tag=f"res{j}") for j in range(4)]
            for j in range(4):
                nc.gpsimd.indirect_dma_start(
                    out=res[j][:], out_offset=None, in_=q_nat[:],
                    in_offset=bass.IndirectOffsetOnAxis(ap=idx_col[j][:, 0:1], axis=0),
                )
            # transpose resampled -> rT (d-part, c-free), bf16
            rT_sb = []
            for d in range(8):
                prt = pcp.tile([128, 512], BF16, name="prt", tag="pc")
                for j in range(4):
                    nc.tensor.transpose(
                        prt[:, 128 * j:128 * (j + 1)],
                        res[j][:, 128 * d:128 * (d + 1)], identb[:],
                    )
                rt = pp.tile([128, 512], BF16, name=f"rT{d}", tag=f"rT{d}")
                nc.vector.tensor_copy(rt[:], prt[:])
                rT_sb.append(rt)

            # rope-q factor: gather fkT rows then transpose into (128, 512)
            pfq = pcp.tile([64, 512], F32, name="pfq", tag="pc")
            for j in range(4):
                fqg = scr.tile([128, 64], F32, name="fqg", tag="fqg")
                nc.gpsimd.indirect_dma_start(
                    out=fqg[:], out_offset=None, in_=fkT[:],
                    in_offset=bass.IndirectOffsetOnAxis(ap=idx_col[j][:, 0:1], axis=0),
                )
                nc.tensor.transpose(pfq[:, 128 * j:128 * (j + 1)], fqg[:],
                                    ident)
            fq_half = scr.tile([64, 512], F32, name="fq_half", tag="fq_half")
            nc.vector.tensor_copy(fq_half[:], pfq[:])
            pfq2 = pcp.tile([128, 512], F32, name="pfq2", tag="pc")
            nc.tensor.matmul(pfq2[:], lhsT=rep64c, rhs=fq_half[:],
                             start=True, stop=True)
            fq_rep = pp.tile([128, 512], F32, name="fq_rep", tag="fq_rep")
            nc.vector.tensor_copy(fq_rep[:], pfq2[:])

            # q-proj (+rope) -> qT_sb
            for e in range(4):
                pq = pcp.tile([128, 512], F32, name="pq", tag="pc")
                for d in range(8):
                    nc.tensor.matmul(
                        pq[:], lhsT=wq_sb[d][:, 128 * e:128 * (e + 1)],
                        rhs=rT_sb[d][:], start=(d == 0), stop=(d == 7),
                    )
                nc.vector.tensor_tensor(qT_sb[e][:], pq[:], fq_rep[:], op=OP.mult)

        # ===== Phase D (second half) =====
        emit_kvproj([2, 3])

        pv_cm.__exit__(None, None, None)
        pk_cm.__exit__(None, None, None)
        d_pools.__exit__(None, None, None)
        prod_cm.__exit__(None, None, None)
        ap_pools.__exit__(None, None, None)

        # ===== Phase E: SDPA (4 waves of 2 heads) =====
        with (
            tc.tile_pool(name="psc", bufs=2, space="PSUM") as pscp,
            tc.tile_pool(name="patt", bufs=2, space="PSUM") as pattp,
            tc.tile_pool(name="epool", bufs=4) as ep,
        ):
            for e in range(4):
                patt = [pattp.tile([65, 512], F32, name=f"patt{hh}", tag=f"patt{hh}") for hh in range(2)]
                for tch in range(32):
                    psc = pscp.tile([128, 1024], F32)
                    for hh in range(2):
                        nc.tensor.matmul(
                            psc[:, 512 * hh:512 * (hh + 1)],
                            lhsT=kT_sb[e][64 * hh:64 * (hh + 1),
                                          128 * tch:128 * (tch + 1)],
                            rhs=qT_sb[e][64 * hh:64 * (hh + 1), :],
                            start=True, stop=True,
                        )
                    et = ep.tile([128, 1024], BF16, name="et", tag="et")
                    nc.scalar.activation(et[:], psc[:], AF.Exp)
                    for hh in range(2):
                        vb = 520 * tch + 65 * (2 * e + hh)
                        nc.tensor.matmul(
                            patt[hh][:],
                            lhsT=v_sb[:, vb:vb + 65],
                            rhs=et[:, 512 * hh:512 * (hh + 1)],
                            start=(tch == 0), stop=(tch == 31),
                        )
                for hh in range(2):
                    recip = scr.tile([1, 512], F32, name="recip", tag="recip", bufs=1)
                    nc.vector.reciprocal(recip[:], patt[hh][64:65, :])
                    rrep = scr.tile([64, 512], F32, name="rrep", tag="rrep", bufs=1)
                    nc.gpsimd.partition_broadcast(rrep[:], recip[:], channels=64)
                    nc.vector.tensor_tensor(
                        att_sb[e][64 * hh:64 * (hh + 1), :],
                        patt[hh][0:64, :], rrep[:], op=OP.mult,
                    )

        # ===== Phase F: out-proj + scale + scatter =====
        with (
            tc.tile_pool(name="po", bufs=2, space="PSUM") as pop,
            tc.tile_pool(name="opool", bufs=2) as op_,
        ):
            for j in range(4):
                po = pop.tile([128, 1024], F32)
                for e in range(4):
                    for k in range(2):
                        nc.tensor.matmul(
                            po[:, 512 * k:512 * (k + 1)],
                            lhsT=att_sb[e][:, 128 * j:128 * (j + 1)],
                            rhs=ow_sb[e][:, 512 * k:512 * (k + 1)],
                            start=(e == 0), stop=(e == 3),
                        )
                osb = op_.tile([128, 1024], F32, name="osb", tag="osb")
                nc.scalar.mul(osb[:], po[:], topw_col[j][:, 2:3])
                nc.sync.dma_start(out_rows[128 * j:128 * (j + 1), 0:D], osb[:])


_NC_CACHE = None


def _get_nc():
    global _NC_CACHE
    if _NC_CACHE is None:
        _NC_CACHE = _build_program()
    return _NC_CACHE


def _host_constants():
    pos = np.arange(S, dtype=np.float32)
    freqs = np.exp(
        np.linspace(0.0, -1.0, dh // 2, dtype=np.float32)
        * np.log(np.float32(ROPE_BASE))
    ).astype(np.float32)
    angles = pos[:, None] * freqs[None, :]          # (S, 32) f32
    fkT = np.concatenate([np.sin(angles), np.cos(angles)], axis=1).astype(
        np.float32
    )                                               # (S, 64)
    fkrep = np.concatenate([fkT.T, fkT.T], axis=0)  # (128, S), pure sin/cos

    p = np.arange(128)[:, None]
    c = np.arange(32)[None, :]
    iota_cm = (128 * c + p).astype(np.float32)

    cblob = np.zeros((128, C_TOT), np.float32)
    cblob[:, C_IDENT:C_IDENT + 128] = np.eye(128, dtype=np.float32)
    cblob[0:32, C_U32:C_U32 + 32] = np.triu(np.ones((32, 32), np.float32))
    cblob[:, C_L128:C_L128 + 128] = np.triu(np.ones((128, 128), np.float32), k=1)
    cblob[0:1, C_ONES:C_ONES + 128] = 1.0
    cblob[0:64, C_REP64:C_REP64 + 128] = np.tile(np.eye(64, dtype=np.float32), (1, 2))

    cblob16 = np.zeros((128, H_TOT), np.float16)
    cblob16[:, H_IOTAH:H_IOTAH + 32] = (iota_cm // 64).astype(np.float16)
    cblob16[:, H_IOTAL:H_IOTAL + 32] = (iota_cm % 64).astype(np.float16)
    cblob16[:, H_IOTA512:H_IOTA512 + 512] = np.tile(
        np.arange(512, dtype=np.float16)[None, :], (128, 1)
    )
    cblob16[:, H_FKREP:H_FKREP + S] = fkrep.astype(np.float16)

    return fkT, cblob, cblob16


def make_in_maps(query_seq, value_seq, router_w, q_w, kv_w, out_w):
    query_seq = np.asarray(query_seq, np.float32)
    value_seq = np.asarray(value_seq, np.float32)
    router_w = np.asarray(router_w, np.float32)
    q_w = np.asarray(q_w, np.float32)
    kv_w = np.asarray(kv_w, np.float32)
    out_w = np.asarray(out_w, np.float32)

    fkT, cblob_base, cblob16 = _host_constants()

    vTs = [_bf16(np.ascontiguousarray(value_seq[b].T)) for b in range(B)]

    walls = []
    for g in range(HG):
        es = slice(EH * g, EH * (g + 1))
        wall = np.zeros((D, 2560), np.float32)
        wall[:, 0:512] = kv_w[es, :].T / 8.0          # 1/sqrt(dh) folded in wk
        wall[:, 512:1024] = kv_w[D + EH * g:D + EH * (g + 1), :].T
        wall[:, 1024:1536] = q_w[es, :].T
        wall[0:512, 1536:2560] = out_w[:, es].T
        walls.append(_bf16(wall))

    cblob = cblob_base.copy()
    cblob[:, C_RW:C_RW + 1024] = router_w.reshape(1, D)

    in_maps = []
    for core in range(8):
        b, g = core // 2, core % 2
        m = dict(
            q_nat=np.ascontiguousarray(query_seq[b]),
            vT=vTs[b],
            wall=walls[g],
            cblob=cblob,
            cblob16=cblob16,
            fkT=fkT,
        )
        in_maps.append(m)
    return in_maps


def assemble_output(outs_by_name):
    """Full (B,S,D) output from per-core results concatenated on axis 0.

    Cores 2b, 2b+1 hold batch b's two head-group partials of the 512
    selected rows (identical selection), scattered into a zero background
    on the host."""
    rows = np.asarray(outs_by_name["out_rows"], dtype=np.float32)
    out = np.zeros((B, S, D), np.float32)
    for b in range(B):
        r0 = rows[(2 * b) * CAP:(2 * b + 1) * CAP]
        r1 = rows[(2 * b + 1) * CAP:(2 * b + 2) * CAP]
        idx = r0[:, D].astype(np.int64)
        out[b, idx] = r0[:, 0:D] + r1[:, 0:D]
    return out


def kernel(query_seq, value_seq, router_w, q_w, kv_w, out_w):
    nc = _get_nc()
    in_maps = make_in_maps(query_seq, value_seq, router_w, q_w, kv_w, out_w)
    try:
        res = run_bass_kernel_spmd(nc, in_maps, list(range(8))).results
    except Exception:
        # transient NRT_EXEC_UNIT_UNRECOVERABLE from a prior wedged session
        # clears on the next dispatch; retry once
        res = run_bass_kernel_spmd(nc, in_maps, list(range(8))).results
    out = np.zeros((B, S, D), np.float32)
    for b in range(B):
        r0 = np.asarray(res[2 * b]["out_rows"], np.float32)
        r1 = np.asarray(res[2 * b + 1]["out_rows"], np.float32)
        idx = r0[:, D].astype(np.int64)
        out[b, idx] = r0[:, 0:D] + r1[:, 0:D]
    return out


# revision 13
# speedup vs baseline: 1.5443x; 1.0470x over previous
"""CapacitiveMHA Trainium2 kernel.

Sharding: 8 cores = 4 batch shards x 2 head-group (tensor-parallel) shards.
Each core handles (batch b, heads [8g, 8g+8)): router+topk+gather replicated
per batch shard; q/kv/out projections and SDPA tensor-parallel over heads;
each core scatters its partial out-projection over the full sequence and the
host sums the two head-group partials per batch.

Router scores run on the DVE (fused multiply-reduce over f32 q rows), so q
is streamed once in natural layout — no host-side transpose or bf16 split.
kv-projection runs on the PE concurrently with the router/top-k (emission
split), and all constants are packed into two blobs to minimize the number
of executable operands.
"""

import sys

sys.path.insert(0, "/opt/trn_rl_repo")

import numpy as np
import ml_dtypes

import concourse.bass as bass
import concourse.bacc as bacc
import concourse.mybir as mybir
import concourse.tile as tile
from concourse.bass_utils import run_bass_kernel_spmd

B, S, D = 4, 4096, 1024
H = 16          # total heads
HG = 2          # head groups (TP degree)
HC = H // HG    # heads per core = 8
dh = D // H     # 64
EH = D // HG    # e-range per core = 512
CAP = 512       # capacity
ROPE_BASE = 10000.0

dt = mybir.dt
F32, BF16, I32 = dt.float32, dt.bfloat16, dt.int32
F16 = dt.float16
AF = mybir.ActivationFunctionType
OP = mybir.AluOpType
X = mybir.AxisListType.X

# cblob (f32) column layout
C_IDENT = 0          # [128, 128] identity
C_U32 = 128          # [32, 32] upper-tri ones (inclusive)
C_L128 = 160         # [128, 128] strict upper-tri ones
C_ONES = 288         # [1, 128] ones row
C_REP64 = 416        # [64, 128] tiled eye(64) twice
C_RW = 544           # [128, 1024] router weights replicated across partitions
C_TOT = 1568

# cblob16 (f16) column layout
H_IOTAH = 0          # [128, 32] (128c+p)//64
H_IOTAL = 32         # [128, 32] (128c+p)%64
H_IOTA512 = 64       # [128, 512] iota row 0..511
H_FKREP = 576        # [128, 4096] k-side rope sin/cos (1/sqrt(d) folded in wk)
H_COMBO = 4672       # [128, 128] interleave(iotah, iotal, 0, 0) quads
H_TOT = 4800


def _bf16(x):
    return np.asarray(x, dtype=ml_dtypes.bfloat16)


def _build_program():
    nc = bacc.Bacc()

    q_nat = nc.dram_tensor("q_nat", [S, D], F32, kind="ExternalInput")
    vT = nc.dram_tensor("vT", [D, S], BF16, kind="ExternalInput")
    wall = nc.dram_tensor("wall", [D, 2560], BF16, kind="ExternalInput")
    cblob = nc.dram_tensor("cblob", [128, C_TOT], F32, kind="ExternalInput")
    cblob16 = nc.dram_tensor("cblob16", [128, H_TOT], F16, kind="ExternalInput")
    fkT = nc.dram_tensor("fkT", [S, dh], F32, kind="ExternalInput")

    out_rows = nc.dram_tensor("out_rows", [CAP, D + 8], F32, kind="ExternalOutput")

    with tile.TileContext(nc) as tc:
        _body(nc, tc, locals())
    nc.compile()
    return nc


def _body(nc, tc, t):
    q_nat, vT, wall, fkT = t["q_nat"], t["vT"], t["wall"], t["fkT"]
    cblob_d, cblob16_d = t["cblob"], t["cblob16"]
    out_rows = t["out_rows"]

    with (
        tc.tile_pool(name="const", bufs=1) as cp,
        tc.tile_pool(name="persist", bufs=1) as pp,
        tc.tile_pool(name="scratch", bufs=2) as scr,
        tc.tile_pool(name="respool", bufs=1) as resp,
    ):
        # ---- constants (two packed blobs) ----
        cb = cp.tile([128, C_TOT], F32, name="cb", tag="cb")
        nc.gpsimd.dma_start(cb[:], cblob_d[:])
        cb16 = cp.tile([128, H_TOT], F16, name="cb16", tag="cb16")

        ident = cb[:, C_IDENT:C_IDENT + 128]
        u32c = cb[0:32, C_U32:C_U32 + 32]
        l128c = cb[:, C_L128:C_L128 + 128]
        ones1x128 = cb[0:1, C_ONES:C_ONES + 128]
        rep64c = cb[0:64, C_REP64:C_REP64 + 128]
        rw_rep = cb[:, C_RW:C_RW + 1024]
        iotah = cb16[:, H_IOTAH:H_IOTAH + 32]
        iotal = cb16[:, H_IOTAL:H_IOTAL + 32]
        iota512 = cb16[:, H_IOTA512:H_IOTA512 + 512]

        identb = cp.tile([128, 128], BF16, name="identb", tag="identb")
        nc.vector.tensor_copy(identb[:], ident)

        # ---- weights from the packed wall tensor ----
        wk_sb, wv_sb, wq_sb, ow_sb = [], [], [], []
        for d in range(8):
            rsl = slice(128 * d, 128 * (d + 1))
            tk = pp.tile([128, EH], BF16, name=f"wk{d}", tag=f"wk{d}")
            nc.gpsimd.dma_start(tk[:], wall[rsl, 0:512])
            wk_sb.append(tk)
            tv = pp.tile([128, EH], BF16, name=f"wv{d}", tag=f"wv{d}")
            nc.gpsimd.dma_start(tv[:], wall[rsl, 512:1024])
            wv_sb.append(tv)
            tq = pp.tile([128, EH], BF16, name=f"wq{d}", tag=f"wq{d}")
            nc.gpsimd.dma_start(tq[:], wall[rsl, 1024:1536])
            wq_sb.append(tq)
        for e in range(4):
            to = pp.tile([128, D], BF16, name=f"ow{e}", tag=f"ow{e}")
            nc.gpsimd.dma_start(to[:], wall[128 * e:128 * (e + 1), 1536:2560])
            ow_sb.append(to)
        nc.gpsimd.dma_start(cb16[:], cblob16_d[:])

        # persistent activations
        kT_sb = [pp.tile([128, S], BF16, name=f"kT{e}", tag=f"kT{e}") for e in range(4)]
        qT_sb = [pp.tile([128, 512], BF16, name=f"qTt{e}", tag=f"qTt{e}") for e in range(4)]
        att_sb = [pp.tile([128, 512], BF16, name=f"att{e}", tag=f"att{e}") for e in range(4)]
        v_sb = pp.tile([128, 520 * 32], BF16, name="v_sb", tag="v_sb")  # 32 chunks x (8h x 65)
        scores_cm = pp.tile([128, 32], F32, name="scores_cm", tag="scores_cm")

        # ===== Phase A: router scores on DVE (q streamed once, f32) =====
        ap_pools = tc.tile_pool(name="qstream", bufs=2)
        qsp = ap_pools.__enter__()
        prod_cm = tc.tile_pool(name="prodp", bufs=2)
        prodp = prod_cm.__enter__()
        for tq4 in range(32):
            qt = qsp.tile([128, D], F32, name="qt", tag="qt")
            nc.sync.dma_start(qt[:], q_nat[128 * tq4:128 * (tq4 + 1), :])
            prod = prodp.tile([128, D], F32, name="prod", tag="prod")
            nc.vector.affine_mul_reduce(
                out=prod[:], accum_out=scores_cm[:, tq4:tq4 + 1],
                in0=qt[:], in1=rw_rep, scale=1.0, bias=0.0,
            )

        # ===== Phase D (first half): kv-proj on PE, overlapped with A/B/C =====
        d_pools = tc.tile_pool(name="stream", bufs=12)
        streamp = d_pools.__enter__()
        pk_cm = tc.tile_pool(name="pk", bufs=2, space="PSUM")
        pkp = pk_cm.__enter__()
        pv_cm = tc.tile_pool(name="pv", bufs=2, space="PSUM")
        pvp = pv_cm.__enter__()

        def emit_kvproj(sc2_list):
            for sc2 in sc2_list:
                vblk = []
                for d in range(8):
                    blk = streamp.tile([128, 1024], BF16, name="stream",
                                       tag="stream")
                    nc.sync.dma_start(
                        blk[:],
                        vT[128 * d:128 * (d + 1), 1024 * sc2:1024 * (sc2 + 1)],
                    )
                    vblk.append(blk)
                for sch in range(2):
                    sc = 2 * sc2 + sch
                    for e in range(4):
                        pk = pkp.tile([128, 512], F32)
                        for d in range(8):
                            nc.tensor.matmul(
                                pk[:], lhsT=wk_sb[d][:, 128 * e:128 * (e + 1)],
                                rhs=vblk[d][:, 512 * sch:512 * (sch + 1)],
                                start=(d == 0), stop=(d == 7),
                            )
                        nc.vector.tensor_tensor(
                            kT_sb[e][:, 512 * sc:512 * (sc + 1)], pk[:],
                            cb16[:, H_FKREP + 512 * sc:H_FKREP + 512 * (sc + 1)],
                            op=OP.mult,
                        )
                    for q4 in range(4):
                        pv = pvp.tile([128, 512], F32)
                        for d in range(8):
                            nc.tensor.matmul(
                                pv[:],
                                lhsT=vblk[d][:, 512 * sch + 128 * q4:
                                             512 * sch + 128 * (q4 + 1)],
                                rhs=wv_sb[d][:], start=(d == 0), stop=(d == 7),
                            )
                        base = 520 * (4 * sc + q4)
                        nc.vector.tensor_copy(
                            v_sb[:, base:base + 520]
                            .rearrange("p (h c) -> p h c", h=8)[:, :, 0:64],
                            pv[:].rearrange("p (h c) -> p h c", h=8),
                        )
                        nc.vector.memset(
                            v_sb[:, base:base + 520]
                            .rearrange("p (h c) -> p h c", h=8)[:, :, 64:65],
                            1.0,
                        )

        emit_kvproj([0, 1, 2])

        # ===== Phase B: top-512 threshold + compaction =====
        # The selection chain runs on Pool/DVE (concurrent with kv-proj on
        # PE); only the cross-partition prefix, one-hot extraction and
        # column transposes touch the PE, emitted after the kv-proj queue.
        res = [resp.tile([128, D], BF16, name=f"res{j}", tag=f"res{j}") for j in range(4)]
        fqg = [scr.tile([128, 64], F32, name=f"fqg{j}", tag=f"fqg{j}", bufs=1) for j in range(4)]
        with tc.tile_pool(name="pb", bufs=2, space="PSUM") as pbp:
            kth = scr.tile([1, 2], F32, name="kth", tag="kth")
            # quantile s.t. k_adj = floor((1-q)*4095) = 510 -> out[0,1] =
            # desc[511] = the 512th-largest score = selection threshold
            nc.gpsimd.kth_largest(
                kth[:], scores_cm[:], 32, 510, quantile=1.0 - 510.5 / 4095.0
            )
            thr_col = scr.tile([128, 1], F32, name="thr_col", tag="thr_col")
            nc.gpsimd.partition_broadcast(thr_col[:], kth[:, 1:2], channels=128)

            mask = pp.tile([128, 32], F32, name="mask", tag="mask")
            nc.vector.tensor_scalar(mask[:], scores_cm[:], thr_col[:], None,
                                    op0=OP.is_ge)
            # in-row inclusive prefix along c (per partition)
            pfx = scr.tile([128, 32], F32, name="pfx", tag="pfx")
            nc.vector.tensor_tensor_scan(pfx[:], mask[:], mask[:], 0.0,
                                         op0=OP.add, op1=OP.bypass)

            # fp16 split of scores: s = shi + slo (each fp16-exact);
            # depends only on scores, runs concurrent with kth on Pool
            shi = scr.tile([128, 32], F16, name="shi", tag="shi")
            nc.vector.tensor_copy(shi[:], scores_cm[:])
            slo = scr.tile([128, 32], F16, name="slo", tag="slo")
            nc.vector.tensor_tensor(slo[:], scores_cm[:], shi[:], op=OP.subtract)

            # combo tile: [ihi | ilo | shi | slo] quads; iota halves come
            # precomputed from cblob16
            combo = scr.tile([128, 128], F16, name="combo", tag="combo")
            nc.vector.tensor_copy(combo[:], cb16[:, H_COMBO:H_COMBO + 128])
            for ci, srct in enumerate((shi[:], slo[:])):
                nc.vector.tensor_copy(
                    combo[:].rearrange("p (c four) -> p c four", four=4)
                    [:, :, ci + 2:ci + 3],
                    srct.rearrange("p (c one) -> p c one", one=1),
                )

            # cross-row exclusive prefix of row totals (one PE op)
            pS = pbp.tile([128, 1], F32, name="pS", tag="pb")
            nc.tensor.matmul(pS[:], lhsT=l128c, rhs=pfx[:, 31:32],
                             start=True, stop=True)
            Scol = scr.tile([128, 1], F32, name="Scol", tag="Scol")
            nc.vector.tensor_copy(Scol[:], pS[:])

            # rank = pfx + Scol - mask; rank_eff = mask ? rank : 512
            rank = scr.tile([128, 32], F32, name="rank", tag="rank")
            nc.vector.scalar_tensor_tensor(rank[:], pfx[:], Scol[:], mask[:],
                                           op0=OP.add, op1=OP.subtract)
            nc.vector.scalar_tensor_tensor(rank[:], rank[:], float(CAP), mask[:],
                                           op0=OP.subtract, op1=OP.mult)
            rank16 = scr.tile([128, 32], F16, name="rank16", tag="rank16")
            nc.vector.tensor_scalar(rank16[:], rank[:], float(CAP), None,
                                    op0=OP.add)

            # one-hot P tiles + [idx_hi; idx_lo; w_hi; w_lo] extraction
            piw = pbp.tile([4, 512], F32, name="piw", tag="pb")
            for c in range(32):
                Pc = scr.tile([128, 512], F16, name="Pc", tag="Pc")
                nc.vector.tensor_tensor(
                    Pc[:], rank16[:, c:c + 1].to_broadcast([128, 512]), iota512,
                    op=OP.is_equal,
                )
                nc.tensor.matmul(piw[:], lhsT=combo[:, 4 * c:4 * c + 4], rhs=Pc[:],
                                 start=(c == 0), stop=(c == 31))
            iw_sb = scr.tile([4, 512], F32, name="iw_sb", tag="iw_sb")
            nc.vector.tensor_copy(iw_sb[:], piw[:])

            # transpose to column layout; kick each gather as soon as its
            # index column lands
            idx_col = []
            topw_col = []
            for j in range(4):
                pt = pbp.tile([128, 4], F32, name="pt", tag="pb")
                nc.tensor.transpose(pt[:], iw_sb[:, 128 * j:128 * (j + 1)],
                                    ident[:4, :4])
                iwT = pp.tile([128, 4], F32, name=f"iwT{j}", tag=f"iwT{j}")
                nc.vector.tensor_copy(iwT[:], pt[:])
                # idx = 64*hi + lo ; w = whi + wlo
                nc.vector.scalar_tensor_tensor(iwT[:, 0:1], iwT[:, 0:1], 64.0,
                                               iwT[:, 1:2], op0=OP.mult,
                                               op1=OP.add)
                nc.vector.tensor_tensor(iwT[:, 2:3], iwT[:, 2:3], iwT[:, 3:4],
                                        op=OP.add)
                ic = pp.tile([128, 1], I32, name=f"idxc{j}", tag=f"idxc{j}")
                nc.vector.tensor_copy(ic[:], iwT[:, 0:1])
                nc.gpsimd.dma_start(
                    out_rows[128 * j:128 * (j + 1), D:D + 1], iwT[:, 0:1])
                idx_col.append(ic)
                topw_col.append(iwT)
                nc.gpsimd.indirect_dma_start(
                    out=res[j][:], out_offset=None, in_=q_nat[:],
                    in_offset=bass.IndirectOffsetOnAxis(ap=ic[:, 0:1], axis=0),
                )
                nc.gpsimd.indirect_dma_start(
                    out=fqg[j][:], out_offset=None, in_=fkT[:],
                    in_offset=bass.IndirectOffsetOnAxis(ap=ic[:, 0:1], axis=0),
                )

        emit_kvproj([3])

        # ===== Phase C: q-proj + rope-q on the pre-gathered rows =====
        with tc.tile_pool(name="pc", bufs=2, space="PSUM") as pcp:
            # transpose resampled -> rT (d-part, c-free), bf16
            rT_sb = []
            for d in range(8):
                prt = pcp.tile([128, 512], BF16, name="prt", tag="pc")
                for j in range(4):
                    nc.tensor.transpose(
                        prt[:, 128 * j:128 * (j + 1)],
                        res[j][:, 128 * d:128 * (d + 1)], identb[:],
                    )
                rt = pp.tile([128, 512], BF16, name=f"rT{d}", tag=f"rT{d}")
                nc.vector.tensor_copy(rt[:], prt[:])
                rT_sb.append(rt)

            # rope-q factor: transpose pre-gathered fkT rows into (128, 512)
            pfq = pcp.tile([64, 512], F32, name="pfq", tag="pc")
            for j in range(4):
                nc.tensor.transpose(pfq[:, 128 * j:128 * (j + 1)], fqg[j][:],
                                    ident)
            fq_half = scr.tile([64, 512], F32, name="fq_half", tag="fq_half")
            nc.vector.tensor_copy(fq_half[:], pfq[:])
            pfq2 = pcp.tile([128, 512], F32, name="pfq2", tag="pc")
            nc.tensor.matmul(pfq2[:], lhsT=rep64c, rhs=fq_half[:],
                             start=True, stop=True)
            fq_rep = pp.tile([128, 512], F32, name="fq_rep", tag="fq_rep")
            nc.vector.tensor_copy(fq_rep[:], pfq2[:])

            # q-proj (+rope) -> qT_sb
            for e in range(4):
                pq = pcp.tile([128, 512], F32, name="pq", tag="pc")
                for d in range(8):
                    nc.tensor.matmul(
                        pq[:], lhsT=wq_sb[d][:, 128 * e:128 * (e + 1)],
                        rhs=rT_sb[d][:], start=(d == 0), stop=(d == 7),
                    )
                nc.vector.tensor_tensor(qT_sb[e][:], pq[:], fq_rep[:], op=OP.mult)

        pv_cm.__exit__(None, None, None)
        pk_cm.__exit__(None, None, None)
        d_pools.__exit__(None, None, None)
        prod_cm.__exit__(None, None, None)
        ap_pools.__exit__(None, None, None)

        # ===== Phase E: SDPA (4 waves of 2 heads) =====
        with (
            tc.tile_pool(name="psc", bufs=2, space="PSUM") as pscp,
            tc.tile_pool(name="patt", bufs=2, space="PSUM") as pattp,
            tc.tile_pool(name="epool", bufs=4) as ep,
        ):
            for e in range(4):
                patt = [pattp.tile([65, 512], F32, name=f"patt{hh}", tag=f"patt{hh}") for hh in range(2)]
                for tch in range(32):
                    psc = pscp.tile([128, 1024], F32)
                    for hh in range(2):
                        nc.tensor.matmul(
                            psc[:, 512 * hh:512 * (hh + 1)],
                            lhsT=kT_sb[e][64 * hh:64 * (hh + 1),
                                          128 * tch:128 * (tch + 1)],
                            rhs=qT_sb[e][64 * hh:64 * (hh + 1), :],
                            start=True, stop=True,
                        )
                    et = ep.tile([128, 1024], BF16, name="et", tag="et")
                    nc.scalar.activation(et[:], psc[:], AF.Exp)
                    for hh in range(2):
                        vb = 520 * tch + 65 * (2 * e + hh)
                        nc.tensor.matmul(
                            patt[hh][:],
                            lhsT=v_sb[:, vb:vb + 65],
                            rhs=et[:, 512 * hh:512 * (hh + 1)],
                            start=(tch == 0), stop=(tch == 31),
                        )
                for hh in range(2):
                    recip = scr.tile([1, 512], F32, name="recip", tag="recip", bufs=1)
                    nc.vector.reciprocal(recip[:], patt[hh][64:65, :])
                    rrep = scr.tile([64, 512], F32, name="rrep", tag="rrep", bufs=1)
                    nc.gpsimd.partition_broadcast(rrep[:], recip[:], channels=64)
                    nc.vector.tensor_tensor(
                        att_sb[e][64 * hh:64 * (hh + 1), :],
                        patt[hh][0:64, :], rrep[:], op=OP.mult,
                    )

        # ===== Phase F: out-proj + scale + scatter =====
        with (
            tc.tile_pool(name="po", bufs=2, space="PSUM") as pop,
            tc.tile_pool(name="opool", bufs=2) as op_,
        ):
            for j in range(4):
                po = pop.tile([128, 1024], F32)
                for e in range(4):
                    for k in range(2):
                        nc.tensor.matmul(
                            po[:, 512 * k:512 * (k + 1)],
                            lhsT=att_sb[e][:, 128 * j:128 * (j + 1)],
                            rhs=ow_sb[e][:, 512 * k:512 * (k + 1)],
                            start=(e == 0), stop=(e == 3),
                        )
                osb = op_.tile([128, 1024], F32, name="osb", tag="osb")
                nc.scalar.mul(osb[:], po[:], topw_col[j][:, 2:3])
                nc.sync.dma_start(out_rows[128 * j:128 * (j + 1), 0:D], osb[:])


_NC_CACHE = None


def _get_nc():
    global _NC_CACHE
    if _NC_CACHE is None:
        _NC_CACHE = _build_program()
    return _NC_CACHE


def _host_constants():
    pos = np.arange(S, dtype=np.float32)
    freqs = np.exp(
        np.linspace(0.0, -1.0, dh // 2, dtype=np.float32)
        * np.log(np.float32(ROPE_BASE))
    ).astype(np.float32)
    angles = pos[:, None] * freqs[None, :]          # (S, 32) f32
    fkT = np.concatenate([np.sin(angles), np.cos(angles)], axis=1).astype(
        np.float32
    )                                               # (S, 64)
    fkrep = np.concatenate([fkT.T, fkT.T], axis=0)  # (128, S), pure sin/cos

    p = np.arange(128)[:, None]
    c = np.arange(32)[None, :]
    iota_cm = (128 * c + p).astype(np.float32)

    cblob = np.zeros((128, C_TOT), np.float32)
    cblob[:, C_IDENT:C_IDENT + 128] = np.eye(128, dtype=np.float32)
    cblob[0:32, C_U32:C_U32 + 32] = np.triu(np.ones((32, 32), np.float32))
    cblob[:, C_L128:C_L128 + 128] = np.triu(np.ones((128, 128), np.float32), k=1)
    cblob[0:1, C_ONES:C_ONES + 128] = 1.0
    cblob[0:64, C_REP64:C_REP64 + 128] = np.tile(np.eye(64, dtype=np.float32), (1, 2))

    cblob16 = np.zeros((128, H_TOT), np.float16)
    cblob16[:, H_IOTAH:H_IOTAH + 32] = (iota_cm // 64).astype(np.float16)
    cblob16[:, H_IOTAL:H_IOTAL + 32] = (iota_cm % 64).astype(np.float16)
    cblob16[:, H_IOTA512:H_IOTA512 + 512] = np.tile(
        np.arange(512, dtype=np.float16)[None, :], (128, 1)
    )
    cblob16[:, H_FKREP:H_FKREP + S] = fkrep.astype(np.float16)
    combo_base = np.zeros((128, 32, 4), np.float16)
    combo_base[:, :, 0] = (iota_cm // 64).astype(np.float16)
    combo_base[:, :, 1] = (iota_cm % 64).astype(np.float16)
    cblob16[:, H_COMBO:H_COMBO + 128] = combo_base.reshape(128, 128)

    return fkT, cblob, cblob16


def make_in_maps(query_seq, value_seq, router_w, q_w, kv_w, out_w):
    query_seq = np.asarray(query_seq, np.float32)
    value_seq = np.asarray(value_seq, np.float32)
    router_w = np.asarray(router_w, np.float32)
    q_w = np.asarray(q_w, np.float32)
    kv_w = np.asarray(kv_w, np.float32)
    out_w = np.asarray(out_w, np.float32)

    fkT, cblob_base, cblob16 = _host_constants()

    vTs = [_bf16(np.ascontiguousarray(value_seq[b].T)) for b in range(B)]

    walls = []
    for g in range(HG):
        es = slice(EH * g, EH * (g + 1))
        wall = np.zeros((D, 2560), np.float32)
        wall[:, 0:512] = kv_w[es, :].T / 8.0          # 1/sqrt(dh) folded in wk
        wall[:, 512:1024] = kv_w[D + EH * g:D + EH * (g + 1), :].T
        wall[:, 1024:1536] = q_w[es, :].T
        wall[0:512, 1536:2560] = out_w[:, es].T
        walls.append(_bf16(wall))

    cblob = cblob_base.copy()
    cblob[:, C_RW:C_RW + 1024] = router_w.reshape(1, D)

    in_maps = []
    for core in range(8):
        b, g = core // 2, core % 2
        m = dict(
            q_nat=np.ascontiguousarray(query_seq[b]),
            vT=vTs[b],
            wall=walls[g],
            cblob=cblob,
            cblob16=cblob16,
            fkT=fkT,
        )
        in_maps.append(m)
    return in_maps


def assemble_output(outs_by_name):
    """Full (B,S,D) output from per-core results concatenated on axis 0.

    Cores 2b, 2b+1 hold batch b's two head-group partials of the 512
    selected rows (identical selection), scattered into a zero background
    on the host."""
    rows = np.asarray(outs_by_name["out_rows"], dtype=np.float32)
    out = np.zeros((B, S, D), np.float32)
    for b in range(B):
        r0 = rows[(2 * b) * CAP:(2 * b + 1) * CAP]
        r1 = rows[(2 * b + 1) * CAP:(2 * b + 2) * CAP]
        idx = r0[:, D].astype(np.int64)
        out[b, idx] = r0[:, 0:D] + r1[:, 0:D]
    return out


def kernel(query_seq, value_seq, router_w, q_w, kv_w, out_w):
    nc = _get_nc()
    in_maps = make_in_maps(query_seq, value_seq, router_w, q_w, kv_w, out_w)
    try:
        res = run_bass_kernel_spmd(nc, in_maps, list(range(8))).results
    except Exception:
        # transient NRT_EXEC_UNIT_UNRECOVERABLE from a prior wedged session
        # clears on the next dispatch; retry once
        res = run_bass_kernel_spmd(nc, in_maps, list(range(8))).results
    out = np.zeros((B, S, D), np.float32)
    for b in range(B):
        r0 = np.asarray(res[2 * b]["out_rows"], np.float32)
        r1 = np.asarray(res[2 * b + 1]["out_rows"], np.float32)
        idx = r0[:, D].astype(np.int64)
        out[b, idx] = r0[:, 0:D] + r1[:, 0:D]
    return out
